# revision 35
# baseline (speedup 1.0000x reference)
"""Multi-head attention (B=4, S=2048, HID=1024, H=16, D=64) on 8 trn2 cores.

Sharding: batch x head-group (4 x 2). Core (2b+g) owns batch b and heads
8g..8g+7 over the FULL sequence: Q/K/V projections for its 8 heads,
attention, and a partial o-projection over its 512 value features. The host
sums the two partial o outputs per batch (the "all-reduce after o_proj"
done host-side) -- no duplicated projection work, no collectives.

Per-core dataflow (all matmuls full 128-partition moving operands, bf16,
fp32 PSUM accumulate -- avoids the half-bandwidth 64-partition moving path
and PE tiling-mode-switch drains):
  - K.T per pair packed [128=2x64 feat, token] bf16
  - Q.T per head zero-padded to [128, token] bf16 (other head's rows = 0),
    so logits contract over 128 partitions with the packed K stationary
  - V' in [token, (kt, head, 65)] bf16 with a ones column per head
    (softmax denominator falls out of the AV matmul as row 64)
  - logits L.T[k, q] in PSUM [128, 1024] (2 k-tiles); exp on ScalarE
  - AV accumulates vals'[65, 512] over 16 k-tiles; row 64 = denominator
  - denominator rows DMA'd from PSUM into a partition-major [8, 512] tile;
    ONE reciprocal per pair (free-size bound: 8x cheaper than reciprocal of
    broadcast tiles); bounced via DRAM back to a flat row, PE-broadcast,
    DVE multiply into vn
  - o_proj tail: vn (bf16) @ w_o.T shard (bf16) over 4 feature chunks
"""
import sys
sys.path.insert(0, "/opt/trn_rl_repo")
import numpy as np

import concourse.bass as bass
import concourse.mybir as mybir
import concourse.tile as tile
from concourse import bacc
from concourse.bass_utils import run_bass_kernel_spmd

F32 = mybir.dt.float32
F32R = mybir.dt.float32r
BF16 = mybir.dt.bfloat16
EXP = mybir.ActivationFunctionType.Exp

B, S, HID, H, D = 4, 2048, 1024, 16, 64
G = 2                  # head groups (cores per batch)
HG = H // G            # 8 heads per core
NPAIR = HG // 2        # 4 head pairs per core
HT = HID // 128        # 8 hid contraction tiles
TB = S // 512          # 4 proj token blocks
KT = S // 128          # 16 key-token tiles
QB = S // 512          # 4 query blocks of 512
N_CORES = 8


def build_nc(n_iter: int = 1):
    nc = bacc.Bacc(None, target_bir_lowering=False)

    xt = nc.dram_tensor("xt", [HID, S], BF16, kind="ExternalInput")
    wq = nc.dram_tensor("wq", [NPAIR * HID, 128], BF16, kind="ExternalInput")
    wk = nc.dram_tensor("wk", [NPAIR * HID, 128], BF16, kind="ExternalInput")
    wv = nc.dram_tensor("wv", [HID, HG * D], BF16, kind="ExternalInput")
    wo = nc.dram_tensor("wo", [HG * D, HID], BF16, kind="ExternalInput")
    cone8 = nc.dram_tensor("cone8", [128, 8], BF16, kind="ExternalInput")
    o = nc.dram_tensor("o", [S, HID], F32, kind="ExternalOutput")

    with tile.TileContext(nc) as tc:
        def body():
            with (
                tc.tile_pool(name="const", bufs=1) as constp,
                tc.tile_pool(name="xtp", bufs=1) as xtp,
                tc.tile_pool(name="vtp", bufs=1) as vtp,
                tc.tile_pool(name="vnp", bufs=1) as vnp,
                tc.tile_pool(name="wop", bufs=1) as wop,
                tc.tile_pool(name="ktqp", bufs=1) as ktqp,
            ):
                ones8_sb = constp.tile([128, 8], BF16)
                nc.sync.dma_start(ones8_sb[:], cone8[:])

                xt_sb = [xtp.tile([128, S], BF16, name=f"xt{t}") for t in range(HT)]
                for t in range(HT):
                    nc.sync.dma_start(xt_sb[t][:], xt[128 * t:128 * (t + 1), :])
                wo_sb = [wop.tile([128, HID], BF16, name=f"wo{c}") for c in range(NPAIR)]
                for c in range(NPAIR):
                    nc.sync.dma_start(wo_sb[c][:], wo[128 * c:128 * (c + 1), :])

                # V' [token, (kt, head, 65)] bf16, resident in SBUF
                vt = vtp.tile([128, KT * HG * 65], BF16)
                vt4 = vt.rearrange("p (t h c) -> p t h c", h=HG, c=65)
                # normalized values [feat(128=2 heads), pair-chunk, token]
                vn_all = vnp.tile([128, NPAIR * S], BF16)

                # persistent K/Q tiles, double-buffered across pairs.
                # qt_h zero-halves are memset once and never overwritten.
                kt_t = [ktqp.tile([128, S], BF16, name=f"kt{i}") for i in range(2)]
                qt_t = [[ktqp.tile([128, S], BF16, name=f"qt{i}{h2}") for h2 in range(2)]
                        for i in range(2)]
                for i in range(2):
                    nc.any.memset(qt_t[i][0][64:128, :], 0.0)
                    nc.any.memset(qt_t[i][1][0:64, :], 0.0)

                with (
                    tc.tile_pool(name="wvp", bufs=1) as wvp,
                    tc.tile_pool(name="wkp", bufs=2) as wkp,
                    tc.tile_pool(name="wqp", bufs=2) as wqp,
                    tc.tile_pool(name="ptp", bufs=3) as ptp,
                    tc.tile_pool(name="vap", bufs=18) as vap,
                    tc.tile_pool(name="nrm", bufs=2) as nrm,
                    tc.tile_pool(name="dramp", bufs=3, space="DRAM") as dramp,
                    tc.tile_pool(name="psP", bufs=2, space="PSUM") as psP,
                    tc.tile_pool(name="psL", bufs=2, space="PSUM") as psL,
                    tc.tile_pool(name="psV", bufs=2, space="PSUM") as psV,
                ):
                    wv_sb = [wvp.tile([128, HG * D], BF16, name=f"wv{t}") for t in range(HT)]
                    for t in range(HT):
                        nc.sync.dma_start(wv_sb[t][:], wv[128 * t:128 * (t + 1), :])

                    # prewarm the exp table while input DMAs are in flight
                    warm = nrm.tile([1, 8], BF16, tag="warm")
                    nc.scalar.activation(warm[:], ones8_sb[0:1, :], EXP, scale=0.0)

                    def v_proj(tokt):
                        vps = psP.tile([128, 512], F32, tag="pp")
                        for ht in range(HT):
                            nc.tensor.matmul(
                                vps[:],
                                xt_sb[ht][:, 128 * tokt:128 * (tokt + 1)],
                                wv_sb[ht][:],
                                start=(ht == 0), stop=(ht == HT - 1),
                            )
                        nc.vector.tensor_copy(
                            vt4[:, tokt, :, 0:64],
                            vps.rearrange("p (h c) -> p h c", c=64),
                        )
                        nc.vector.tensor_copy(vt4[:, tokt, :, 64], ones8_sb[:])

                    # ---- pair pipeline: K/Q proj + attention + normalize ----
                    va_tiles = {}

                    def proj_pair(j, w_dram, pool, evac):
                        w_p = pool.tile([128, HID], BF16, tag="wp")
                        nc.sync.dma_start(
                            w_p.rearrange("p (t c) -> p t c", c=128),
                            w_dram[HID * j:HID * (j + 1), :].rearrange("(t p) c -> p t c", p=128),
                        )
                        for tb in range(TB):
                            pps = psP.tile([128, 512], F32, tag="pp")
                            for ht in range(HT):
                                nc.tensor.matmul(
                                    pps[:],
                                    w_p[:, 128 * ht:128 * (ht + 1)],
                                    xt_sb[ht][:, 512 * tb:512 * (tb + 1)],
                                    start=(ht == 0), stop=(ht == HT - 1),
                                )
                            evac(tb, pps)

                    def attn_block(j, h2, qb, kt_sb, qt_sb, den_flat, v_inter=False):
                        h = 2 * j + h2
                        vals = psV.tile([65, 512], F32, tag="vv")
                        for ktp2 in range(KT // 2):
                            lg = psL.tile([128, 1024], F32, tag="lg")
                            for u in range(2):
                                kt = 2 * ktp2 + u
                                nc.tensor.matmul(
                                    lg[:, 512 * u:512 * (u + 1)],
                                    kt_sb[:, 128 * kt:128 * (kt + 1)],
                                    qt_sb[:, 512 * qb:512 * (qb + 1)],
                                    start=True, stop=True,
                                )
                            pt = ptp.tile([128, 1024], BF16, tag="pt")
                            nc.scalar.activation(pt[:], lg[:], EXP, scale=0.125)
                            if v_inter:
                                # first attention block: V' projection for these
                                # two k-tiles lands just ahead of their AV use
                                v_proj(2 * ktp2)
                                v_proj(2 * ktp2 + 1)
                            for u in range(2):
                                kt = 2 * ktp2 + u
                                nc.tensor.matmul(
                                    vals[:],
                                    vt4[:, kt, h, :],
                                    pt[:, 512 * u:512 * (u + 1)],
                                    start=(ktp2 == 0 and u == 0),
                                    stop=(ktp2 == KT // 2 - 1 and u == 1),
                                )
                        nc.vector.tensor_copy(den_flat[0:1, 512 * qb:512 * (qb + 1)], vals[64:65, :])
                        va = vap.tile([64, 512], BF16, tag="va")
                        nc.vector.tensor_copy(va[:], vals[0:64, :])
                        va_tiles[8 * j + 4 * h2 + qb] = va

                    rec_drams = {}

                    def chain(j, h2, den_flat, use_act=False):
                        # reciprocal of this half-pair's 4 denominator rows.
                        # engine ops cannot address partitions 1..31, so either
                        # bounce through DRAM to partition-major for the DVE
                        # reciprocal, or (for the last chain, when the scalar
                        # engine has gone idle) reciprocal the flat row on ACT.
                        rec_dram = dramp.tile([4, 512], BF16, tag="rdram")
                        den_dram = dramp.tile([4, 512], BF16, tag="ddram")
                        nc.sync.dma_start(
                            den_dram.rearrange("r c -> (r c)")[None, :], den_flat[0:1, :]
                        )
                        den_sq = nrm.tile([4, 512], BF16, tag="dsq")
                        nc.sync.dma_start(den_sq[:], den_dram[:])
                        rec_sq = nrm.tile([4, 512], BF16, tag="rsq")
                        with nc.allow_low_precision(reason="denominator reciprocal in bf16"):
                            nc.vector.reciprocal(rec_sq[:], den_sq[:])
                        nc.sync.dma_start(rec_dram[:], rec_sq[:])
                        rec_drams[(j, h2)] = rec_dram

                    def normalize(j, h2):
                        # runs a half-pair or more late: the reciprocal chain
                        # has had a full attention half to complete, so nothing
                        # here blocks the in-order engine streams
                        rec_dram = rec_drams.pop((j, h2))
                        for qb in range(QB):
                            bcs = nrm.tile([64, 512], BF16, tag="bcs")
                            nc.sync.dma_start(
                                bcs[:], rec_dram[qb:qb + 1, :].broadcast_to([64, 512])
                            )
                            nc.vector.tensor_mul(
                                vn_all[64 * h2:64 * (h2 + 1),
                                       S * j + 512 * qb:S * j + 512 * (qb + 1)],
                                va_tiles.pop(8 * j + 4 * h2 + qb)[:],
                                bcs[:],
                            )

                    def k_evac_f(kt_sb):
                        def k_evac(tb, pps):
                            nc.vector.tensor_copy(kt_sb[:, 512 * tb:512 * (tb + 1)], pps[:])
                        return k_evac

                    def q_evac_f(jj):
                        def q_evac(tb, pps):
                            nc.vector.tensor_copy(
                                qt_t[jj % 2][0][0:64, 512 * tb:512 * (tb + 1)], pps[0:64, :])
                            nc.vector.tensor_copy(
                                qt_t[jj % 2][1][64:128, 512 * tb:512 * (tb + 1)], pps[64:128, :])
                        return q_evac

                    proj_pair(0, wk, wkp, k_evac_f(kt_t[0]))
                    proj_pair(0, wq, wqp, q_evac_f(0))
                    for j in range(NPAIR):
                        kt_sb = kt_t[j % 2]
                        for h2 in range(2):
                            den_flat = nrm.tile([1, 4 * 512], BF16, tag="dflat")
                            for qb in range(QB):
                                attn_block(j, h2, qb, kt_sb, qt_t[j % 2][h2], den_flat,
                                           v_inter=(j == 0 and h2 == 0 and qb == 0))
                            chain(j, h2, den_flat, use_act=(j == NPAIR - 1 and h2 == 1))
                        normalize(j, 0)
                        if j < NPAIR - 1:
                            proj_pair(j + 1, wk, wkp, k_evac_f(kt_t[(j + 1) % 2]))
                            proj_pair(j + 1, wq, wqp, q_evac_f(j + 1))
                        normalize(j, 1)

                # ---- o projection tail ----
                with (
                    tc.tile_pool(name="obp", bufs=2) as obp,
                    tc.tile_pool(name="psO", bufs=2, space="PSUM") as psO,
                ):
                    vn3 = vn_all.rearrange("p (c s) -> p c s", c=NPAIR)
                    for tokb in range(S // 128):
                        o_sb = obp.tile([128, HID], F32)
                        for ob in range(2):
                            ops = psO.tile([128, 512], F32, tag="oo")
                            for c in range(NPAIR):
                                nc.tensor.matmul(
                                    ops[:],
                                    vn3[:, c, 128 * tokb:128 * (tokb + 1)],
                                    wo_sb[c][:, 512 * ob:512 * (ob + 1)],
                                    start=(c == 0), stop=(c == NPAIR - 1),
                                )
                            nc.vector.tensor_copy(o_sb[:, 512 * ob:512 * (ob + 1)], ops[:])
                        nc.sync.dma_start(o[128 * tokb:128 * (tokb + 1), :], o_sb[:])

        if n_iter > 1:
            with tc.For_i(0, n_iter, 1):
                body()
        else:
            body()

    nc.compile()
    return nc


def shard_inputs(x, w_qkv, w_o):
    x = np.asarray(x, dtype=np.float32)
    w_qkv = np.asarray(w_qkv, dtype=np.float32)
    w_o = np.asarray(w_o, dtype=np.float32)
    import ml_dtypes
    bf = ml_dtypes.bfloat16

    # w_qkv row (h*192 + c): c<64 q, 64<=c<128 k, 128<=c<192 v
    w3 = w_qkv.reshape(H, 3 * D, HID)
    wq_h = w3[:, 0:D, :]        # [H, D, HID]
    wk_h = w3[:, D:2 * D, :]
    wv_h = w3[:, 2 * D:3 * D, :]
    wo_t = w_o.T                # [HID(vals feat, h-major), HID(out)]

    cone8 = np.ones((128, 8), np.float32).astype(bf)
    in_maps = []
    for core in range(N_CORES):
        b, g = core // G, core % G
        hsel = slice(HG * g, HG * (g + 1))
        wq_g = wq_h[hsel].reshape(NPAIR, 2 * D, HID).transpose(0, 2, 1).reshape(NPAIR * HID, 128)
        wk_g = wk_h[hsel].reshape(NPAIR, 2 * D, HID).transpose(0, 2, 1).reshape(NPAIR * HID, 128)
        wv_g = wv_h[hsel].reshape(HG * D, HID).T        # [HID, 512]
        wo_g = wo_t[HG * D * g:HG * D * (g + 1), :]     # [512, HID]
        in_maps.append({
            "xt": np.ascontiguousarray(x[b].T).astype(bf),
            "wq": np.ascontiguousarray(wq_g).astype(bf),
            "wk": np.ascontiguousarray(wk_g).astype(bf),
            "wv": np.ascontiguousarray(wv_g).astype(bf),
            "wo": np.ascontiguousarray(wo_g).astype(bf),
            "cone8": cone8,
        })
    return in_maps


_NC_CACHE = {}


def get_nc(n_iter: int = 1):
    if n_iter not in _NC_CACHE:
        _NC_CACHE[n_iter] = build_nc(n_iter)
    return _NC_CACHE[n_iter]


def kernel(x, w_qkv, w_o):
    nc = get_nc(1)
    in_maps = shard_inputs(x, w_qkv, w_o)
    res = run_bass_kernel_spmd(nc, in_maps, list(range(N_CORES)))
    out = np.empty((B, S, HID), np.float32)
    for b in range(B):
        out[b] = res.results[G * b]["o"]
        for g in range(1, G):
            out[b] += res.results[G * b + g]["o"]
    return out


# revision 40
# speedup vs baseline: 1.0357x; 1.0357x over previous
"""Multi-head attention (B=4, S=2048, HID=1024, H=16, D=64) on 8 trn2 cores.

Sharding: batch x head-group (4 x 2). Core (2b+g) owns batch b and heads
8g..8g+7 over the FULL sequence: Q/K/V projections for its 8 heads,
attention, and a partial o-projection over its 512 value features. The host
sums the two partial o outputs per batch (the "all-reduce after o_proj"
done host-side) -- no duplicated projection work, no collectives.

Per-core dataflow (all matmuls full 128-partition moving operands, bf16,
fp32 PSUM accumulate -- avoids the half-bandwidth 64-partition moving path
and PE tiling-mode-switch drains):
  - K.T per pair packed [128=2x64 feat, token] bf16
  - Q.T per head zero-padded to [128, token] bf16 (other head's rows = 0),
    so logits contract over 128 partitions with the packed K stationary
  - V' in [token, (kt, head, 65)] bf16 with a ones column per head
    (softmax denominator falls out of the AV matmul as row 64)
  - logits L.T[k, q] in PSUM [128, 1024] (2 k-tiles); exp on ScalarE
  - AV accumulates vals'[65, 512] over 16 k-tiles; row 64 = denominator
  - denominator rows DMA'd from PSUM into a partition-major [8, 512] tile;
    ONE reciprocal per pair (free-size bound: 8x cheaper than reciprocal of
    broadcast tiles); bounced via DRAM back to a flat row, PE-broadcast,
    DVE multiply into vn
  - o_proj tail: vn (bf16) @ w_o.T shard (bf16) over 4 feature chunks
"""
import sys
sys.path.insert(0, "/opt/trn_rl_repo")
import numpy as np

import concourse.bass as bass
import concourse.mybir as mybir
import concourse.tile as tile
from concourse import bacc
from concourse.bass_utils import run_bass_kernel_spmd

F32 = mybir.dt.float32
F32R = mybir.dt.float32r
BF16 = mybir.dt.bfloat16
EXP = mybir.ActivationFunctionType.Exp

B, S, HID, H, D = 4, 2048, 1024, 16, 64
G = 2                  # head groups (cores per batch)
HG = H // G            # 8 heads per core
NPAIR = HG // 2        # 4 head pairs per core
HT = HID // 128        # 8 hid contraction tiles
TB = S // 512          # 4 proj token blocks
KT = S // 128          # 16 key-token tiles
QB = S // 512          # 4 query blocks of 512
N_CORES = 8


def build_nc(n_iter: int = 1):
    nc = bacc.Bacc(None, target_bir_lowering=False)

    xt = nc.dram_tensor("xt", [HID, S], BF16, kind="ExternalInput")
    wq = nc.dram_tensor("wq", [NPAIR * HID, 128], BF16, kind="ExternalInput")
    wk = nc.dram_tensor("wk", [NPAIR * HID, 128], BF16, kind="ExternalInput")
    wv = nc.dram_tensor("wv", [HID, HG * D], BF16, kind="ExternalInput")
    wo = nc.dram_tensor("wo", [HG * D, HID], BF16, kind="ExternalInput")
    cone8 = nc.dram_tensor("cone8", [128, 8], BF16, kind="ExternalInput")
    o = nc.dram_tensor("o", [S, HID], F32, kind="ExternalOutput")

    with tile.TileContext(nc) as tc:
        def body():
            with (
                tc.tile_pool(name="const", bufs=1) as constp,
                tc.tile_pool(name="xtp", bufs=1) as xtp,
                tc.tile_pool(name="vtp", bufs=1) as vtp,
                tc.tile_pool(name="vnp", bufs=1) as vnp,
                tc.tile_pool(name="wop", bufs=1) as wop,
                tc.tile_pool(name="ktqp", bufs=1) as ktqp,
            ):
                ones8_sb = constp.tile([128, 8], BF16)
                nc.sync.dma_start(ones8_sb[:], cone8[:])

                xt_sb = [xtp.tile([128, S], BF16, name=f"xt{t}") for t in range(HT)]
                wo_sb = [wop.tile([128, HID], BF16, name=f"wo{c}") for c in range(NPAIR)]

                # V' [token, (kt, head, 65)] bf16, resident in SBUF
                vt = vtp.tile([128, KT * HG * 65], BF16)
                vt4 = vt.rearrange("p (t h c) -> p t h c", h=HG, c=65)
                # normalized values [feat(128=2 heads), pair-chunk, token]
                vn_all = vnp.tile([128, NPAIR * S], BF16)

                # persistent K/Q tiles, double-buffered across pairs.
                # qt_h zero-halves are memset once and never overwritten.
                kt_t = [ktqp.tile([128, S], BF16, name=f"kt{i}") for i in range(2)]
                qt_t = [[ktqp.tile([128, S], BF16, name=f"qt{i}{h2}") for h2 in range(2)]
                        for i in range(2)]
                for i in range(2):
                    nc.any.memset(qt_t[i][0][64:128, :], 0.0)
                    nc.any.memset(qt_t[i][1][0:64, :], 0.0)

                with (
                    tc.tile_pool(name="wvp", bufs=1) as wvp,
                    tc.tile_pool(name="wkp", bufs=2) as wkp,
                    tc.tile_pool(name="wqp", bufs=2) as wqp,
                    tc.tile_pool(name="ptp", bufs=3) as ptp,
                    tc.tile_pool(name="vap", bufs=18) as vap,
                    tc.tile_pool(name="nrm", bufs=2) as nrm,
                    tc.tile_pool(name="dramp", bufs=3, space="DRAM") as dramp,
                    tc.tile_pool(name="psP", bufs=2, space="PSUM") as psP,
                    tc.tile_pool(name="psL", bufs=2, space="PSUM") as psL,
                    tc.tile_pool(name="psV", bufs=2, space="PSUM") as psV,
                ):
                    def proj_dma(j, w_dram, pool):
                        w_p = pool.tile([128, HID], BF16, tag="wp")
                        nc.sync.dma_start(
                            w_p.rearrange("p (t c) -> p t c", c=128),
                            w_dram[HID * j:HID * (j + 1), :].rearrange("(t p) c -> p t c", p=128),
                        )
                        return w_p

                    # DMA queue order = emission order: pair-0 weights first so
                    # the first projection isn't stuck behind 6MB of x/wv/wo
                    wk0 = proj_dma(0, wk, wkp)
                    wq0 = proj_dma(0, wq, wqp)
                    for t in range(HT):
                        nc.sync.dma_start(xt_sb[t][:], xt[128 * t:128 * (t + 1), :])
                    wv_sb = [wvp.tile([128, HG * D], BF16, name=f"wv{t}") for t in range(HT)]
                    for t in range(HT):
                        nc.sync.dma_start(wv_sb[t][:], wv[128 * t:128 * (t + 1), :])
                    for c in range(NPAIR):
                        nc.sync.dma_start(wo_sb[c][:], wo[128 * c:128 * (c + 1), :])

                    # prewarm the activation tables while input DMAs are in
                    # flight -- Ln first pins the natural_log_exp_and_others
                    # set, which also serves every Exp (no mid-kernel reload)
                    warm = nrm.tile([1, 8], BF16, tag="warm")
                    nc.scalar.activation(warm[:], ones8_sb[0:1, :],
                                         mybir.ActivationFunctionType.Ln)
                    nc.scalar.activation(warm[:], ones8_sb[0:1, :], EXP, scale=0.0)

                    def v_proj(tokt):
                        vps = psP.tile([128, 512], F32, tag="pp")
                        for ht in range(HT):
                            nc.tensor.matmul(
                                vps[:],
                                xt_sb[ht][:, 128 * tokt:128 * (tokt + 1)],
                                wv_sb[ht][:],
                                start=(ht == 0), stop=(ht == HT - 1),
                            )
                        nc.vector.tensor_copy(
                            vt4[:, tokt, :, 0:64],
                            vps.rearrange("p (h c) -> p h c", c=64),
                        )
                        nc.vector.tensor_copy(vt4[:, tokt, :, 64], ones8_sb[:])

                    # ---- pair pipeline: K/Q proj + attention + normalize ----
                    va_tiles = {}

                    def proj_pair(w_p, evac):
                        for tb in range(TB):
                            pps = psP.tile([128, 512], F32, tag="pp")
                            for ht in range(HT):
                                nc.tensor.matmul(
                                    pps[:],
                                    w_p[:, 128 * ht:128 * (ht + 1)],
                                    xt_sb[ht][:, 512 * tb:512 * (tb + 1)],
                                    start=(ht == 0), stop=(ht == HT - 1),
                                )
                            evac(tb, pps)

                    def attn_block(j, h2, qb, kt_sb, qt_sb, den_flat, v_inter=False):
                        h = 2 * j + h2
                        vals = psV.tile([65, 512], F32, tag="vv")
                        for ktp2 in range(KT // 2):
                            lg = psL.tile([128, 1024], F32, tag="lg")
                            for u in range(2):
                                kt = 2 * ktp2 + u
                                nc.tensor.matmul(
                                    lg[:, 512 * u:512 * (u + 1)],
                                    kt_sb[:, 128 * kt:128 * (kt + 1)],
                                    qt_sb[:, 512 * qb:512 * (qb + 1)],
                                    start=True, stop=True,
                                )
                            pt = ptp.tile([128, 1024], BF16, tag="pt")
                            nc.scalar.activation(pt[:], lg[:], EXP, scale=0.125)
                            if v_inter:
                                # first attention block: V' projection for these
                                # two k-tiles lands just ahead of their AV use
                                v_proj(2 * ktp2)
                                v_proj(2 * ktp2 + 1)
                            for u in range(2):
                                kt = 2 * ktp2 + u
                                nc.tensor.matmul(
                                    vals[:],
                                    vt4[:, kt, h, :],
                                    pt[:, 512 * u:512 * (u + 1)],
                                    start=(ktp2 == 0 and u == 0),
                                    stop=(ktp2 == KT // 2 - 1 and u == 1),
                                )
                        nc.vector.tensor_copy(den_flat[0:1, 512 * qb:512 * (qb + 1)], vals[64:65, :])
                        va = vap.tile([64, 512], BF16, tag="va")
                        nc.vector.tensor_copy(va[:], vals[0:64, :])
                        va_tiles[8 * j + 4 * h2 + qb] = va

                    rec_drams = {}

                    def chain(j, h2, den_flat, use_act=False):
                        # reciprocal of this half-pair's 4 denominator rows.
                        # engine ops cannot address partitions 1..31, so either
                        # bounce through DRAM to partition-major for the DVE
                        # reciprocal, or (for the last chain, when the scalar
                        # engine has gone idle) reciprocal the flat row on ACT.
                        rec_dram = dramp.tile([4, 512], BF16, tag="rdram")
                        if use_act:
                            # 1/x = exp(-ln x): two ACT ops on the flat row --
                            # the scalar engine is idle by the last chain and
                            # this skips two DMA bounce hops on the o-proj
                            # critical path (both fns live in the preloaded
                            # natural_log_exp table set)
                            lnt = nrm.tile([1, 4 * 512], BF16, tag="lnt")
                            nc.scalar.activation(lnt[:], den_flat[:],
                                                 mybir.ActivationFunctionType.Ln)
                            rec_flat = nrm.tile([1, 4 * 512], BF16, tag="rflat")
                            nc.scalar.activation(rec_flat[:], lnt[:], EXP, scale=-1.0)
                            nc.sync.dma_start(
                                rec_dram.rearrange("r c -> (r c)")[None, :], rec_flat[0:1, :]
                            )
                        else:
                            den_dram = dramp.tile([4, 512], BF16, tag="ddram")
                            nc.sync.dma_start(
                                den_dram.rearrange("r c -> (r c)")[None, :], den_flat[0:1, :]
                            )
                            den_sq = nrm.tile([4, 512], BF16, tag="dsq")
                            nc.sync.dma_start(den_sq[:], den_dram[:])
                            rec_sq = nrm.tile([4, 512], BF16, tag="rsq")
                            with nc.allow_low_precision(reason="denominator reciprocal in bf16"):
                                nc.vector.reciprocal(rec_sq[:], den_sq[:])
                            nc.sync.dma_start(rec_dram[:], rec_sq[:])
                        rec_drams[(j, h2)] = rec_dram

                    def normalize(j, h2):
                        # runs a half-pair or more late: the reciprocal chain
                        # has had a full attention half to complete, so nothing
                        # here blocks the in-order engine streams
                        rec_dram = rec_drams.pop((j, h2))
                        for qb in range(QB):
                            bcs = nrm.tile([64, 512], BF16, tag="bcs")
                            nc.sync.dma_start(
                                bcs[:], rec_dram[qb:qb + 1, :].broadcast_to([64, 512])
                            )
                            nc.vector.tensor_mul(
                                vn_all[64 * h2:64 * (h2 + 1),
                                       S * j + 512 * qb:S * j + 512 * (qb + 1)],
                                va_tiles.pop(8 * j + 4 * h2 + qb)[:],
                                bcs[:],
                            )

                    def k_evac_f(kt_sb):
                        def k_evac(tb, pps):
                            nc.vector.tensor_copy(kt_sb[:, 512 * tb:512 * (tb + 1)], pps[:])
                        return k_evac

                    def q_evac_f(jj):
                        def q_evac(tb, pps):
                            nc.vector.tensor_copy(
                                qt_t[jj % 2][0][0:64, 512 * tb:512 * (tb + 1)], pps[0:64, :])
                            nc.vector.tensor_copy(
                                qt_t[jj % 2][1][64:128, 512 * tb:512 * (tb + 1)], pps[64:128, :])
                        return q_evac

                    proj_pair(wk0, k_evac_f(kt_t[0]))
                    proj_pair(wq0, q_evac_f(0))
                    for j in range(NPAIR):
                        kt_sb = kt_t[j % 2]
                        for h2 in range(2):
                            den_flat = nrm.tile([1, 4 * 512], BF16, tag="dflat")
                            for qb in range(QB):
                                attn_block(j, h2, qb, kt_sb, qt_t[j % 2][h2], den_flat,
                                           v_inter=(j == 0 and h2 == 0 and qb == 0))
                            chain(j, h2, den_flat, use_act=(j == NPAIR - 1 and h2 == 1))
                        normalize(j, 0)
                        if j < NPAIR - 1:
                            proj_pair(proj_dma(j + 1, wk, wkp), k_evac_f(kt_t[(j + 1) % 2]))
                            proj_pair(proj_dma(j + 1, wq, wqp), q_evac_f(j + 1))
                        normalize(j, 1)

                # ---- o projection tail ----
                with (
                    tc.tile_pool(name="obp", bufs=2) as obp,
                    tc.tile_pool(name="psO", bufs=2, space="PSUM") as psO,
                ):
                    vn3 = vn_all.rearrange("p (c s) -> p c s", c=NPAIR)
                    for tokb in range(S // 128):
                        o_sb = obp.tile([128, HID], F32)
                        for ob in range(2):
                            ops = psO.tile([128, 512], F32, tag="oo")
                            for c in range(NPAIR):
                                nc.tensor.matmul(
                                    ops[:],
                                    vn3[:, c, 128 * tokb:128 * (tokb + 1)],
                                    wo_sb[c][:, 512 * ob:512 * (ob + 1)],
                                    start=(c == 0), stop=(c == NPAIR - 1),
                                )
                            nc.vector.tensor_copy(o_sb[:, 512 * ob:512 * (ob + 1)], ops[:])
                        nc.sync.dma_start(o[128 * tokb:128 * (tokb + 1), :], o_sb[:])

        if n_iter > 1:
            with tc.For_i(0, n_iter, 1):
                body()
        else:
            body()

    nc.compile()
    return nc


def shard_inputs(x, w_qkv, w_o):
    x = np.asarray(x, dtype=np.float32)
    w_qkv = np.asarray(w_qkv, dtype=np.float32)
    w_o = np.asarray(w_o, dtype=np.float32)
    import ml_dtypes
    bf = ml_dtypes.bfloat16

    # w_qkv row (h*192 + c): c<64 q, 64<=c<128 k, 128<=c<192 v
    w3 = w_qkv.reshape(H, 3 * D, HID)
    wq_h = w3[:, 0:D, :]        # [H, D, HID]
    wk_h = w3[:, D:2 * D, :]
    wv_h = w3[:, 2 * D:3 * D, :]
    wo_t = w_o.T                # [HID(vals feat, h-major), HID(out)]

    cone8 = np.ones((128, 8), np.float32).astype(bf)
    in_maps = []
    for core in range(N_CORES):
        b, g = core // G, core % G
        hsel = slice(HG * g, HG * (g + 1))
        wq_g = wq_h[hsel].reshape(NPAIR, 2 * D, HID).transpose(0, 2, 1).reshape(NPAIR * HID, 128)
        wk_g = wk_h[hsel].reshape(NPAIR, 2 * D, HID).transpose(0, 2, 1).reshape(NPAIR * HID, 128)
        wv_g = wv_h[hsel].reshape(HG * D, HID).T        # [HID, 512]
        wo_g = wo_t[HG * D * g:HG * D * (g + 1), :]     # [512, HID]
        in_maps.append({
            "xt": np.ascontiguousarray(x[b].T).astype(bf),
            "wq": np.ascontiguousarray(wq_g).astype(bf),
            "wk": np.ascontiguousarray(wk_g).astype(bf),
            "wv": np.ascontiguousarray(wv_g).astype(bf),
            "wo": np.ascontiguousarray(wo_g).astype(bf),
            "cone8": cone8,
        })
    return in_maps


_NC_CACHE = {}


def get_nc(n_iter: int = 1):
    if n_iter not in _NC_CACHE:
        _NC_CACHE[n_iter] = build_nc(n_iter)
    return _NC_CACHE[n_iter]


def kernel(x, w_qkv, w_o):
    nc = get_nc(1)
    in_maps = shard_inputs(x, w_qkv, w_o)
    res = run_bass_kernel_spmd(nc, in_maps, list(range(N_CORES)))
    out = np.empty((B, S, HID), np.float32)
    for b in range(B):
        out[b] = res.results[G * b]["o"]
        for g in range(1, G):
            out[b] += res.results[G * b + g]["o"]
    return out


# revision 41
# speedup vs baseline: 1.0378x; 1.0020x over previous
"""Multi-head attention (B=4, S=2048, HID=1024, H=16, D=64) on 8 trn2 cores.

Sharding: batch x head-group (4 x 2). Core (2b+g) owns batch b and heads
8g..8g+7 over the FULL sequence: Q/K/V projections for its 8 heads,
attention, and a partial o-projection over its 512 value features. The host
sums the two partial o outputs per batch (the "all-reduce after o_proj"
done host-side) -- no duplicated projection work, no collectives.

Per-core dataflow (all matmuls full 128-partition moving operands, bf16,
fp32 PSUM accumulate -- avoids the half-bandwidth 64-partition moving path
and PE tiling-mode-switch drains):
  - K.T per pair packed [128=2x64 feat, token] bf16
  - Q.T per head zero-padded to [128, token] bf16 (other head's rows = 0),
    so logits contract over 128 partitions with the packed K stationary
  - V' in [token, (kt, head, 65)] bf16 with a ones column per head
    (softmax denominator falls out of the AV matmul as row 64)
  - logits L.T[k, q] in PSUM [128, 1024] (2 k-tiles); exp on ScalarE
  - AV accumulates vals'[65, 512] over 16 k-tiles; row 64 = denominator
  - denominator rows DMA'd from PSUM into a partition-major [8, 512] tile;
    ONE reciprocal per pair (free-size bound: 8x cheaper than reciprocal of
    broadcast tiles); bounced via DRAM back to a flat row, PE-broadcast,
    DVE multiply into vn
  - o_proj tail: vn (bf16) @ w_o.T shard (bf16) over 4 feature chunks
"""
import sys
sys.path.insert(0, "/opt/trn_rl_repo")
import numpy as np

import concourse.bass as bass
import concourse.mybir as mybir
import concourse.tile as tile
from concourse import bacc
from concourse.bass_utils import run_bass_kernel_spmd

F32 = mybir.dt.float32
F32R = mybir.dt.float32r
BF16 = mybir.dt.bfloat16
EXP = mybir.ActivationFunctionType.Exp

B, S, HID, H, D = 4, 2048, 1024, 16, 64
G = 2                  # head groups (cores per batch)
HG = H // G            # 8 heads per core
NPAIR = HG // 2        # 4 head pairs per core
HT = HID // 128        # 8 hid contraction tiles
TB = S // 512          # 4 proj token blocks
KT = S // 128          # 16 key-token tiles
QB = S // 512          # 4 query blocks of 512
N_CORES = 8


def build_nc(n_iter: int = 1):
    nc = bacc.Bacc(None, target_bir_lowering=False)

    xt = nc.dram_tensor("xt", [HID, S], BF16, kind="ExternalInput")
    wq = nc.dram_tensor("wq", [NPAIR * HID, 128], BF16, kind="ExternalInput")
    wk = nc.dram_tensor("wk", [NPAIR * HID, 128], BF16, kind="ExternalInput")
    wv = nc.dram_tensor("wv", [HID, HG * D], BF16, kind="ExternalInput")
    wo = nc.dram_tensor("wo", [HG * D, HID], BF16, kind="ExternalInput")
    cone8 = nc.dram_tensor("cone8", [128, 8], BF16, kind="ExternalInput")
    o = nc.dram_tensor("o", [S, HID], F32, kind="ExternalOutput")

    with tile.TileContext(nc) as tc:
        def body():
            with (
                tc.tile_pool(name="const", bufs=1) as constp,
                tc.tile_pool(name="xtp", bufs=1) as xtp,
                tc.tile_pool(name="vtp", bufs=1) as vtp,
                tc.tile_pool(name="vnp", bufs=1) as vnp,
                tc.tile_pool(name="wop", bufs=1) as wop,
                tc.tile_pool(name="ktqp", bufs=1) as ktqp,
            ):
                ones8_sb = constp.tile([128, 8], BF16)
                nc.sync.dma_start(ones8_sb[:], cone8[:])

                xt_sb = [xtp.tile([128, S], BF16, name=f"xt{t}") for t in range(HT)]
                wo_sb = [wop.tile([128, HID], BF16, name=f"wo{c}") for c in range(NPAIR)]

                # V' [token, (kt, head, 65)] bf16, resident in SBUF
                vt = vtp.tile([128, KT * HG * 65], BF16)
                vt4 = vt.rearrange("p (t h c) -> p t h c", h=HG, c=65)
                # normalized values [feat(128=2 heads), pair-chunk, token]
                vn_all = vnp.tile([128, NPAIR * S], BF16)

                # persistent K/Q tiles, double-buffered across pairs.
                # qt_h zero-halves are memset once and never overwritten.
                kt_t = [ktqp.tile([128, S], BF16, name=f"kt{i}") for i in range(2)]
                qt_t = [[ktqp.tile([128, S], BF16, name=f"qt{i}{h2}") for h2 in range(2)]
                        for i in range(2)]
                for i in range(2):
                    nc.any.memset(qt_t[i][0][64:128, :], 0.0)
                    nc.any.memset(qt_t[i][1][0:64, :], 0.0)

                with (
                    tc.tile_pool(name="wvp", bufs=1) as wvp,
                    tc.tile_pool(name="wkp", bufs=2) as wkp,
                    tc.tile_pool(name="wqp", bufs=2) as wqp,
                    tc.tile_pool(name="ptp", bufs=3) as ptp,
                    tc.tile_pool(name="vap", bufs=18) as vap,
                    tc.tile_pool(name="nrm", bufs=2) as nrm,
                    tc.tile_pool(name="dramp", bufs=3, space="DRAM") as dramp,
                    tc.tile_pool(name="psP", bufs=2, space="PSUM") as psP,
                    tc.tile_pool(name="psL", bufs=2, space="PSUM") as psL,
                    tc.tile_pool(name="psV", bufs=2, space="PSUM") as psV,
                ):
                    def proj_dma(j, w_dram, pool):
                        w_p = pool.tile([128, HID], BF16, tag="wp")
                        nc.sync.dma_start(
                            w_p.rearrange("p (t c) -> p t c", c=128),
                            w_dram[HID * j:HID * (j + 1), :].rearrange("(t p) c -> p t c", p=128),
                        )
                        return w_p

                    # DMA queue order = emission order: pair-0 weights first so
                    # the first projection isn't stuck behind 6MB of x/wv/wo
                    wk0 = proj_dma(0, wk, wkp)
                    wq0 = proj_dma(0, wq, wqp)
                    for t in range(HT):
                        nc.sync.dma_start(xt_sb[t][:], xt[128 * t:128 * (t + 1), :])
                    wv_sb = [wvp.tile([128, HG * D], BF16, name=f"wv{t}") for t in range(HT)]
                    for t in range(HT):
                        nc.sync.dma_start(wv_sb[t][:], wv[128 * t:128 * (t + 1), :])
                    for c in range(NPAIR):
                        nc.sync.dma_start(wo_sb[c][:], wo[128 * c:128 * (c + 1), :])

                    # prewarm the activation tables while input DMAs are in
                    # flight -- Ln first pins the natural_log_exp_and_others
                    # set, which also serves every Exp (no mid-kernel reload)
                    warm = nrm.tile([1, 8], BF16, tag="warm")
                    nc.scalar.activation(warm[:], ones8_sb[0:1, :],
                                         mybir.ActivationFunctionType.Ln)
                    nc.scalar.activation(warm[:], ones8_sb[0:1, :], EXP, scale=0.0)

                    def v_proj(tokt):
                        vps = psP.tile([128, 512], F32, tag="pp")
                        for ht in range(HT):
                            nc.tensor.matmul(
                                vps[:],
                                xt_sb[ht][:, 128 * tokt:128 * (tokt + 1)],
                                wv_sb[ht][:],
                                start=(ht == 0), stop=(ht == HT - 1),
                            )
                        nc.vector.tensor_copy(
                            vt4[:, tokt, :, 0:64],
                            vps.rearrange("p (h c) -> p h c", c=64),
                        )
                        nc.vector.tensor_copy(vt4[:, tokt, :, 64], ones8_sb[:])

                    # ---- pair pipeline: K/Q proj + attention + normalize ----
                    va_tiles = {}

                    def proj_pair(w_p, evac):
                        for tb in range(TB):
                            pps = psP.tile([128, 512], F32, tag="pp")
                            for ht in range(HT):
                                nc.tensor.matmul(
                                    pps[:],
                                    w_p[:, 128 * ht:128 * (ht + 1)],
                                    xt_sb[ht][:, 512 * tb:512 * (tb + 1)],
                                    start=(ht == 0), stop=(ht == HT - 1),
                                )
                            evac(tb, pps)

                    def attn_block(j, h2, qb, kt_sb, qt_sb, den_flat, v_inter=False):
                        h = 2 * j + h2
                        vals = psV.tile([65, 512], F32, tag="vv")
                        for ktp2 in range(KT // 2):
                            lg = psL.tile([128, 1024], F32, tag="lg")
                            for u in range(2):
                                kt = 2 * ktp2 + u
                                nc.tensor.matmul(
                                    lg[:, 512 * u:512 * (u + 1)],
                                    kt_sb[:, 128 * kt:128 * (kt + 1)],
                                    qt_sb[:, 512 * qb:512 * (qb + 1)],
                                    start=True, stop=True,
                                )
                            pt = ptp.tile([128, 1024], BF16, tag="pt")
                            nc.scalar.activation(pt[:], lg[:], EXP, scale=0.125)
                            if v_inter:
                                # first attention block: V' projection for these
                                # two k-tiles lands just ahead of their AV use
                                v_proj(2 * ktp2)
                                v_proj(2 * ktp2 + 1)
                            for u in range(2):
                                kt = 2 * ktp2 + u
                                nc.tensor.matmul(
                                    vals[:],
                                    vt4[:, kt, h, :],
                                    pt[:, 512 * u:512 * (u + 1)],
                                    start=(ktp2 == 0 and u == 0),
                                    stop=(ktp2 == KT // 2 - 1 and u == 1),
                                )
                        nc.vector.tensor_copy(den_flat[0:1, 512 * qb:512 * (qb + 1)], vals[64:65, :])
                        va = vap.tile([64, 512], BF16, tag="va")
                        nc.vector.tensor_copy(va[:], vals[0:64, :])
                        va_tiles[8 * j + 4 * h2 + qb] = va

                    rec_drams = {}

                    def chain(j, h2, den_flat, use_act=False):
                        # reciprocal of this half-pair's 4 denominator rows.
                        # engine ops cannot address partitions 1..31, so either
                        # bounce through DRAM to partition-major for the DVE
                        # reciprocal, or (for the last chain, when the scalar
                        # engine has gone idle) reciprocal the flat row on ACT.
                        rec_dram = dramp.tile([4, 512], BF16, tag="rdram")
                        if use_act:
                            # 1/x = exp(-ln x): two ACT ops on the flat row --
                            # the scalar engine is idle by the last chain and
                            # this skips two DMA bounce hops on the o-proj
                            # critical path (both fns live in the preloaded
                            # natural_log_exp table set)
                            lnt = nrm.tile([1, 4 * 512], F32, tag="lnt")
                            nc.scalar.activation(lnt[:], den_flat[:],
                                                 mybir.ActivationFunctionType.Ln)
                            rec_flat = nrm.tile([1, 4 * 512], BF16, tag="rflat")
                            nc.scalar.activation(rec_flat[:], lnt[:], EXP, scale=-1.0)
                            nc.sync.dma_start(
                                rec_dram.rearrange("r c -> (r c)")[None, :], rec_flat[0:1, :]
                            )
                        else:
                            den_dram = dramp.tile([4, 512], BF16, tag="ddram")
                            nc.sync.dma_start(
                                den_dram.rearrange("r c -> (r c)")[None, :], den_flat[0:1, :]
                            )
                            den_sq = nrm.tile([4, 512], BF16, tag="dsq")
                            nc.sync.dma_start(den_sq[:], den_dram[:])
                            rec_sq = nrm.tile([4, 512], BF16, tag="rsq")
                            with nc.allow_low_precision(reason="denominator reciprocal in bf16"):
                                nc.vector.reciprocal(rec_sq[:], den_sq[:])
                            nc.sync.dma_start(rec_dram[:], rec_sq[:])
                        rec_drams[(j, h2)] = rec_dram

                    def normalize(j, h2):
                        # runs a half-pair or more late: the reciprocal chain
                        # has had a full attention half to complete, so nothing
                        # here blocks the in-order engine streams
                        rec_dram = rec_drams.pop((j, h2))
                        for qb in range(QB):
                            bcs = nrm.tile([64, 512], BF16, tag="bcs")
                            nc.sync.dma_start(
                                bcs[:], rec_dram[qb:qb + 1, :].broadcast_to([64, 512])
                            )
                            nc.vector.tensor_mul(
                                vn_all[64 * h2:64 * (h2 + 1),
                                       S * j + 512 * qb:S * j + 512 * (qb + 1)],
                                va_tiles.pop(8 * j + 4 * h2 + qb)[:],
                                bcs[:],
                            )

                    def k_evac_f(kt_sb):
                        def k_evac(tb, pps):
                            nc.vector.tensor_copy(kt_sb[:, 512 * tb:512 * (tb + 1)], pps[:])
                        return k_evac

                    def q_evac_f(jj):
                        def q_evac(tb, pps):
                            nc.vector.tensor_copy(
                                qt_t[jj % 2][0][0:64, 512 * tb:512 * (tb + 1)], pps[0:64, :])
                            nc.vector.tensor_copy(
                                qt_t[jj % 2][1][64:128, 512 * tb:512 * (tb + 1)], pps[64:128, :])
                        return q_evac

                    proj_pair(wk0, k_evac_f(kt_t[0]))
                    proj_pair(wq0, q_evac_f(0))
                    for j in range(NPAIR):
                        kt_sb = kt_t[j % 2]
                        for h2 in range(2):
                            den_flat = nrm.tile([1, 4 * 512], BF16, tag="dflat")
                            for qb in range(QB):
                                attn_block(j, h2, qb, kt_sb, qt_t[j % 2][h2], den_flat,
                                           v_inter=(j == 0 and h2 == 0 and qb == 0))
                            chain(j, h2, den_flat, use_act=(j == NPAIR - 1 and h2 == 1))
                        normalize(j, 0)
                        if j < NPAIR - 1:
                            proj_pair(proj_dma(j + 1, wk, wkp), k_evac_f(kt_t[(j + 1) % 2]))
                            proj_pair(proj_dma(j + 1, wq, wqp), q_evac_f(j + 1))
                        normalize(j, 1)

                # ---- o projection tail ----
                with (
                    tc.tile_pool(name="obp", bufs=2) as obp,
                    tc.tile_pool(name="psO", bufs=2, space="PSUM") as psO,
                ):
                    vn3 = vn_all.rearrange("p (c s) -> p c s", c=NPAIR)
                    for tokb in range(S // 128):
                        o_sb = obp.tile([128, HID], F32)
                        for ob in range(2):
                            ops = psO.tile([128, 512], F32, tag="oo")
                            for c in range(NPAIR):
                                nc.tensor.matmul(
                                    ops[:],
                                    vn3[:, c, 128 * tokb:128 * (tokb + 1)],
                                    wo_sb[c][:, 512 * ob:512 * (ob + 1)],
                                    start=(c == 0), stop=(c == NPAIR - 1),
                                )
                            nc.vector.tensor_copy(o_sb[:, 512 * ob:512 * (ob + 1)], ops[:])
                        nc.sync.dma_start(o[128 * tokb:128 * (tokb + 1), :], o_sb[:])

        if n_iter > 1:
            with tc.For_i(0, n_iter, 1):
                body()
        else:
            body()

    nc.compile()
    return nc


def shard_inputs(x, w_qkv, w_o):
    x = np.asarray(x, dtype=np.float32)
    w_qkv = np.asarray(w_qkv, dtype=np.float32)
    w_o = np.asarray(w_o, dtype=np.float32)
    import ml_dtypes
    bf = ml_dtypes.bfloat16

    # w_qkv row (h*192 + c): c<64 q, 64<=c<128 k, 128<=c<192 v
    w3 = w_qkv.reshape(H, 3 * D, HID)
    wq_h = w3[:, 0:D, :]        # [H, D, HID]
    wk_h = w3[:, D:2 * D, :]
    wv_h = w3[:, 2 * D:3 * D, :]
    wo_t = w_o.T                # [HID(vals feat, h-major), HID(out)]

    cone8 = np.ones((128, 8), np.float32).astype(bf)
    in_maps = []
    for core in range(N_CORES):
        b, g = core // G, core % G
        hsel = slice(HG * g, HG * (g + 1))
        wq_g = wq_h[hsel].reshape(NPAIR, 2 * D, HID).transpose(0, 2, 1).reshape(NPAIR * HID, 128)
        wk_g = wk_h[hsel].reshape(NPAIR, 2 * D, HID).transpose(0, 2, 1).reshape(NPAIR * HID, 128)
        wv_g = wv_h[hsel].reshape(HG * D, HID).T        # [HID, 512]
        wo_g = wo_t[HG * D * g:HG * D * (g + 1), :]     # [512, HID]
        in_maps.append({
            "xt": np.ascontiguousarray(x[b].T).astype(bf),
            "wq": np.ascontiguousarray(wq_g).astype(bf),
            "wk": np.ascontiguousarray(wk_g).astype(bf),
            "wv": np.ascontiguousarray(wv_g).astype(bf),
            "wo": np.ascontiguousarray(wo_g).astype(bf),
            "cone8": cone8,
        })
    return in_maps


_NC_CACHE = {}


def get_nc(n_iter: int = 1):
    if n_iter not in _NC_CACHE:
        _NC_CACHE[n_iter] = build_nc(n_iter)
    return _NC_CACHE[n_iter]


def kernel(x, w_qkv, w_o):
    nc = get_nc(1)
    in_maps = shard_inputs(x, w_qkv, w_o)
    res = run_bass_kernel_spmd(nc, in_maps, list(range(N_CORES)))
    out = np.empty((B, S, HID), np.float32)
    for b in range(B):
        out[b] = res.results[G * b]["o"]
        for g in range(1, G):
            out[b] += res.results[G * b + g]["o"]
    return out
